# revision 40
# baseline (speedup 1.0000x reference)
"""Trainium2 Bass kernel for GRU(I=8,H=6) + Linear(6->4) over [B=4096, T=512].

v4-good fallback: single chain per core, fp16 matmuls, zb-weights,
u2-on-DVE, hm-trick on DVE, po cast on ACT, batched x DMA.
Measured 1,402,095 ns, rel err 8.07e-4, PASS.
"""

import os
import sys

for _p in ("/opt/trn_rl_repo", "/root/.axon_site/_ro/trn_rl_repo"):
    if os.path.isdir(_p) and _p not in sys.path:
        sys.path.insert(0, _p)

import numpy as np

I, H, O = 8, 6, 4
B, T = 4096, 512
NCORES = 8
BS = B // NCORES
G = 4
CB = BS // G            # 128
GH = G * H              # 24
GI = G * I              # 32
GO = G * O              # 16

_CACHE = {}


def _build_module():
    import concourse.tile as tile
    from concourse import bacc, mybir
    from contextlib import ExitStack

    f16 = mybir.dt.float16
    f32 = mybir.dt.float32
    Sig = mybir.ActivationFunctionType.Sigmoid
    Tanh = mybir.ActivationFunctionType.Tanh
    mult = mybir.AluOpType.mult
    add = mybir.AluOpType.add
    subtract = mybir.AluOpType.subtract

    nc = bacc.Bacc(
        "TRN2",
        target_bir_lowering=False,
        debug=False,
        enable_asserts=False,
        num_devices=NCORES,
    )

    xt_d = nc.dram_tensor("xt", [T // 4, GI, 4 * CB], f16, kind="ExternalInput").ap()
    wx_d = nc.dram_tensor("wx", [GI, 128], f16, kind="ExternalInput").ap()
    wh_d = nc.dram_tensor("wh", [GH + 1, 128], f16, kind="ExternalInput").ap()
    wlin_d = nc.dram_tensor("wlin", [GH + 1, GO], f16, kind="ExternalInput").ap()
    hinit_d = nc.dram_tensor("hinit", [GH + 1, CB], f16, kind="ExternalInput").ap()
    out_d = nc.dram_tensor("out", [T // 4, GO, 4 * CB], f16, kind="ExternalOutput").ap()

    with tile.TileContext(nc) as tc, ExitStack() as ctx:
        const = ctx.enter_context(tc.tile_pool(name="const", bufs=1))
        xpool = ctx.enter_context(tc.tile_pool(name="x", bufs=3))
        ps_pool = ctx.enter_context(tc.tile_pool(name="ps", bufs=2, space="PSUM"))
        po_pool = ctx.enter_context(tc.tile_pool(name="po", bufs=2, space="PSUM"))
        rz_pool = ctx.enter_context(tc.tile_pool(name="rz", bufs=3))
        u_pool = ctx.enter_context(tc.tile_pool(name="u", bufs=3))
        n_pool = ctx.enter_context(tc.tile_pool(name="n", bufs=3))
        q_pool = ctx.enter_context(tc.tile_pool(name="q", bufs=3))
        hm_pool = ctx.enter_context(tc.tile_pool(name="hm", bufs=3))
        tb_pool = ctx.enter_context(tc.tile_pool(name="tb", bufs=3))
        po_sb_pool = ctx.enter_context(tc.tile_pool(name="posb", bufs=2))
        hpool = ctx.enter_context(tc.tile_pool(name="h", bufs=1))

        wx_s = const.tile([GI, 128], f16)
        nc.sync.dma_start(wx_s[:], wx_d)
        wh_s = const.tile([GH + 1, 128], f16)
        nc.sync.dma_start(wh_s[:], wh_d)
        wlin_s = const.tile([GH + 1, GO], f16)
        nc.sync.dma_start(wlin_s[:], wlin_d)

        # double-buffered hidden state: h' writes the other buffer each step
        ha = hpool.tile([GH + 1, CB], f16, name="ha", tag="ha")
        nc.sync.dma_start(ha[:], hinit_d)
        hb = hpool.tile([GH + 1, CB], f16, name="hb", tag="hb")
        nc.sync.dma_start(hb[:], hinit_d)
        hbufs = [ha, hb]

        x4 = {}
        ps = {}
        po = None
        po_prev = None
        for tp4 in (0, 1):
            xt = xpool.tile([GI, 4 * CB], f16)
            nc.sync.dma_start(xt[:], xt_d[tp4, :, :])
            x4[tp4] = xt
        p0 = ps_pool.tile([128, CB], f32)
        nc.tensor.matmul(p0[:], wx_s[:], x4[0][:, 0:CB], start=True, stop=False)
        ps[0] = p0

        for t in range(T):
            tt = t % 4
            cur = ps[t]
            h_t = hbufs[t % 2]
            h_new = hbufs[(t + 1) % 2]

            nc.tensor.matmul(cur[:], wh_s[:], h_t[:], start=False, stop=True)

            if t >= 1:
                s = t - 1
                if s % 4 == 0:
                    po = po_pool.tile([GO, 4 * CB], f32, name="po")
                nc.tensor.matmul(
                    po[:, (s % 4) * CB : (s % 4 + 1) * CB], wlin_s[:], h_t[:],
                    start=True, stop=True,
                )
                if s % 4 == 3:
                    po_prev = (po, s // 4)
            if tt == 0 and t // 4 + 2 < T // 4:
                xt = xpool.tile([GI, 4 * CB], f16)
                nc.sync.dma_start(xt[:], xt_d[t // 4 + 2, :, :])
                x4[t // 4 + 2] = xt
            if t + 1 < T:
                t1, k1 = (t + 1) // 4, (t + 1) % 4
                p = ps_pool.tile([128, CB], f32)
                nc.tensor.matmul(
                    p[:], wx_s[:], x4[t1][:, k1 * CB : (k1 + 1) * CB],
                    start=True, stop=False,
                )
                ps[t + 1] = p
                if k1 == 3 and t1 - 1 in x4:
                    del x4[t1 - 1]

            rz = rz_pool.tile([64, CB], f16)
            nc.scalar.activation(rz[:], cur[64:128, :], Sig)

            u = u_pool.tile([GH, CB], f16)
            nc.vector.tensor_tensor(
                out=u[:], in0=rz[32 : 32 + GH, :], in1=cur[32 : 32 + GH, :], op=mult
            )
            u2 = u_pool.tile([GH, CB], f16, name="u2")
            nc.vector.tensor_tensor(out=u2[:], in0=u[:], in1=cur[0:GH, :], op=add)

            # q = zb*h, hm = h - q = z*h on GPSIMD (in-order there), both run
            # during u/u2/tanh; keeps the DVE queue to u,u2,t_b,h' only
            q = q_pool.tile([GH, CB], f16)
            nc.gpsimd.tensor_tensor(out=q[:], in0=rz[0:GH, :], in1=h_t[0:GH, :], op=mult)
            hm = hm_pool.tile([GH, CB], f16)
            nc.vector.tensor_tensor(out=hm[:], in0=h_t[0:GH, :], in1=q[:], op=subtract)

            n_ = n_pool.tile([GH, CB], f16)
            nc.scalar.activation(n_[:], u2[:], Tanh)

            if po_prev is not None:
                po_done, blk = po_prev
                po_sb = po_sb_pool.tile([GO, 4 * CB], f16)
                nc.scalar.copy(po_sb[:], po_done[:])
                nc.sync.dma_start(out_d[blk, :, :], po_sb[:])
                po_prev = None

            t_b = tb_pool.tile([GH, CB], f16)
            nc.vector.tensor_tensor(out=t_b[:], in0=n_[:], in1=rz[0:GH, :], op=mult)
            nc.vector.tensor_tensor(out=h_new[0:GH, :], in0=hm[:], in1=t_b[:], op=add)

            del ps[t]

        nc.tensor.matmul(
            po[:, 3 * CB : 4 * CB], wlin_s[:], hbufs[T % 2][:], start=True, stop=True
        )
        po_sb = po_sb_pool.tile([GO, 4 * CB], f16)
        nc.scalar.copy(po_sb[:], po[:])
        nc.sync.dma_start(out_d[T // 4 - 1, :, :], po_sb[:])

    nc.compile()
    return nc


def _pack_weights(W_ih, W_hh, b_ih, b_hh, W_lin, b_lin):
    wx = np.zeros((GI, 128), np.float32)
    wh = np.zeros((GH + 1, 128), np.float32)
    wlin = np.zeros((GH + 1, GO), np.float32)
    for g in range(G):
        sx = slice(g * I, (g + 1) * I)
        sh = slice(g * H, (g + 1) * H)
        wx[sx, 0 + g * H : 0 + (g + 1) * H] = W_ih[12:18].T
        wh[GH, 0 + g * H : 0 + (g + 1) * H] = b_ih[12:18]
        wh[sh, 32 + g * H : 32 + (g + 1) * H] = W_hh[12:18].T
        wh[GH, 32 + g * H : 32 + (g + 1) * H] = b_hh[12:18]
        wx[sx, 64 + g * H : 64 + (g + 1) * H] = -W_ih[6:12].T
        wh[sh, 64 + g * H : 64 + (g + 1) * H] = -W_hh[6:12].T
        wh[GH, 64 + g * H : 64 + (g + 1) * H] = -(b_ih[6:12] + b_hh[6:12])
        wx[sx, 96 + g * H : 96 + (g + 1) * H] = W_ih[0:6].T
        wh[sh, 96 + g * H : 96 + (g + 1) * H] = W_hh[0:6].T
        wh[GH, 96 + g * H : 96 + (g + 1) * H] = b_ih[0:6] + b_hh[0:6]
        wlin[sh, g * O : (g + 1) * O] = W_lin.T
        wlin[GH, g * O : (g + 1) * O] = b_lin
    return (
        wx.astype(np.float16),
        wh.astype(np.float16),
        wlin.astype(np.float16),
    )


def _run(inputs, trace=False):
    from concourse.bass_utils import run_bass_kernel_spmd

    x = np.ascontiguousarray(np.asarray(inputs["x"], dtype=np.float32))
    W_ih = np.asarray(inputs["W_ih"], np.float32)
    W_hh = np.asarray(inputs["W_hh"], np.float32)
    b_ih = np.asarray(inputs["b_ih"], np.float32)
    b_hh = np.asarray(inputs["b_hh"], np.float32)
    W_lin = np.asarray(inputs["W_lin"], np.float32)
    b_lin = np.asarray(inputs["b_lin"], np.float32)

    if "nc" not in _CACHE:
        _CACHE["nc"] = _build_module()
    nc = _CACHE["nc"]

    wx, wh, wlin = _pack_weights(W_ih, W_hh, b_ih, b_hh, W_lin, b_lin)
    hinit = np.zeros((GH + 1, CB), np.float16)
    hinit[GH, :] = 1.0

    in_maps = []
    for core in range(NCORES):
        xc = x[core * BS : (core + 1) * BS]
        xt = (
            xc.reshape(G, CB, T // 4, 4, I)
            .transpose(2, 0, 4, 3, 1)
            .reshape(T // 4, GI, 4 * CB)
            .astype(np.float16)
        )
        in_maps.append(
            {
                "xt": np.ascontiguousarray(xt),
                "wx": wx,
                "wh": wh,
                "wlin": wlin,
                "hinit": hinit,
            }
        )

    res = run_bass_kernel_spmd(
        nc, in_maps, core_ids=list(range(NCORES)), trace=trace
    )

    outs = []
    for core in range(NCORES):
        a = res.results[core]["out"].astype(np.float32)
        a = a.reshape(T // 4, G, O, 4, CB)
        a = a.transpose(1, 4, 0, 3, 2)
        outs.append(a.reshape(BS, T, O))
    full = np.concatenate(outs, axis=0)
    return full, res


def kernel(**inputs) -> np.ndarray:
    out, _ = _run(inputs, trace=False)
    return out


def kernel_profiled(inputs):
    """Returns (output, BassKernelResults-with-trace)."""
    return _run(inputs, trace=True)
